# revision 5
# baseline (speedup 1.0000x reference)
"""Multi-head attention (B=4, N=2048, DIM=768, H=12) on 8 TRN2 cores.

Sharding: core c -> batch c//2, heads (c%2)*6 .. +6  (6 heads = 3 pairs).
Each core computes its heads' attention and a partial output projection
(row-sharded w_proj); host sums the two partials per batch and adds bias.

Per-core dataflow:
  inputs : xt [768,2048] (= x[b].T), wq/wk/wv [768,384], wp [384,768]
  qkv    : Q^T,K^T per head-pair [128,2048] (d-major), V token-major with a
           ones column appended per head ([64 V | 1] x 6 -> [128, 390])
  scores : S^T[keys, q], 2 heads row-tiled per key-tile matmul
  softmax: exp on ACT (scale folded in; max-subtraction skipped - scores O(5));
           PV with M=65 makes row 64 of U the denominator for free;
           normalization via fast reciprocal + PE-replicated recip rows
  PV     : U^T[d+1, q] accumulated over key chunks (V' chunks stationary)
  proj   : partial = OT.T-slices @ wp  (OT is d-major already - no transposes)

Matmul dtype is configurable: fp32 is exact but runs LOW_HIGH double-pass at
2 cyc/col per pass on the PE; bf16 is 4x faster per matmul.
"""

import sys

for _p in ("/opt/trn_rl_repo",):
    if _p not in sys.path:
        sys.path.insert(0, _p)

import numpy as np
import ml_dtypes

import concourse.bass as bass
import concourse.bacc as bacc
import concourse.mybir as mybir
import concourse.tile as tile
from concourse.bass_utils import run_bass_kernel_spmd
from concourse.masks import make_identity

DIM = 768
HEADS = 12
HD = 64
B = 4
N = 2048
NCORES = 8
PAIRS = 3          # head-pairs per core (6 heads)
CH = DIM // 128    # 6 contraction chunks of 128
KT = N // 128      # 16 key tiles
QB = N // 512      # 4 query blocks of 512
F32 = mybir.dt.float32
EXP = mybir.ActivationFunctionType.Exp
SCALE = HD ** -0.5

DEFAULT_DTYPE = "bf16"


def build_program(dtype="f32", debug=False, overlap=None):
    if overlap is None:
        overlap = dtype != "f32"
    dt = F32 if dtype == "f32" else mybir.dt.bfloat16
    nc = bacc.Bacc()
    xt = nc.declare_dram_parameter("xt", [DIM, N], dt, isOutput=False)
    wq = nc.declare_dram_parameter("wq", [DIM, PAIRS * 128], dt, isOutput=False)
    wk = nc.declare_dram_parameter("wk", [DIM, PAIRS * 128], dt, isOutput=False)
    wv = nc.declare_dram_parameter("wv", [DIM, PAIRS * 128], dt, isOutput=False)
    wp = nc.declare_dram_parameter("wp", [PAIRS * 128, DIM], dt, isOutput=False)
    out = nc.declare_dram_parameter("out", [N, DIM], F32, isOutput=True)
    dbg = None
    if debug:
        dbg = {
            "dbg_v0": nc.declare_dram_parameter("dbg_v0", [128, 390], F32, isOutput=True),
            "dbg_den": nc.declare_dram_parameter("dbg_den", [1, 1024], F32, isOutput=True),
            "dbg_dsb": nc.declare_dram_parameter("dbg_dsb", [1, 1024], F32, isOutput=True),
            "dbg_e0": nc.declare_dram_parameter("dbg_e0", [128, 1024], F32, isOutput=True),
        }

    with tile.TileContext(nc) as tc:
        emit(tc, nc, xt, wq, wk, wv, wp, out, dt, overlap=overlap, dbg=dbg)
    nc.compile()
    return nc


def emit(tc, nc, xt, wq, wk, wv, wp, out, dt, overlap, dbg=None):
    import contextlib

    ctx = contextlib.ExitStack()
    wbufs = 2 if overlap else 1
    with ctx:
        sb = ctx.enter_context(tc.tile_pool(name="sb", bufs=1))
        ps = ctx.enter_context(tc.tile_pool(name="ps", bufs=1, space="PSUM"))

        # ---- load inputs -------------------------------------------------
        # DMA order matches first-consumer order: wq/wk (small) first, then
        # xt split into qb-column blocks so qkv groups unblock progressively,
        # then wv/wp. Tile tracks deps at slice granularity.
        def load_w(dram, nm):
            tiles = []
            for ch in range(CH):
                t = sb.tile([128, PAIRS * 128], dt, name=f"{nm}{ch}", tag=f"{nm}{ch}")
                nc.sync.dma_start(out=t[:], in_=dram[ch * 128:(ch + 1) * 128, :])
                tiles.append(t)
            return tiles

        wq_sb = load_w(wq, "wq")
        wk_sb = load_w(wk, "wk")

        xt_sb = [sb.tile([128, N], dt, name=f"xt{ch}", tag=f"xt{ch}")
                 for ch in range(CH)]
        for qb in range(QB):
            csl = slice(qb * 512, (qb + 1) * 512)
            for ch in range(CH):
                nc.sync.dma_start(out=xt_sb[ch][:, csl],
                                  in_=xt[ch * 128:(ch + 1) * 128, csl])

        wv_sb = load_w(wv, "wv")

        wp_sb = []
        for ch in range(PAIRS):
            t = sb.tile([128, DIM], dt, name=f"wp{ch}", tag=f"wp{ch}")
            nc.sync.dma_start(out=t[:], in_=wp[ch * 128:(ch + 1) * 128, :])
            wp_sb.append(t)

        ones_sb = sb.tile([128, 64], dt, name="ones", tag="ones")
        nc.vector.memset(ones_sb[:], 1.0)
        ident = sb.tile([128, 128], dt, name="ident", tag="ident")
        make_identity(nc, ident)

        # persistent SBUF tensors
        # v' layout per head g: cols [g*65 .. g*65+63] = V, col g*65+64 = 1.0
        v_sb = [sb.tile([128, 6 * 65], dt, name=f"v{k}", tag=f"v{k}")
                for k in range(KT)]
        ot_sb = [sb.tile([128, N], dt, name=f"ot{p}", tag=f"ot{p}")
                 for p in range(PAIRS)]

        # ---- V' = [x @ wv | 1] (token-major) -----------------------------
        for kt in range(KT):
            pv = ps.tile([128, 512], F32, name="dr", tag="dr", bufs=2)
            for ch in range(CH):
                nc.tensor.matmul(
                    pv[:, :PAIRS * 128],
                    lhsT=xt_sb[ch][:, kt * 128:(kt + 1) * 128],
                    rhs=wv_sb[ch][:],
                    start=(ch == 0), stop=(ch == CH - 1),
                )
            v3 = v_sb[kt].rearrange("p (g c) -> p g c", c=65)
            p3 = pv[:, :PAIRS * 128].rearrange("p (g c) -> p g c", c=64)
            nc.vector.tensor_copy(v3[:, :, 0:64], p3[:])
            nc.vector.memset(v3[:, :, 64:65], 1.0)
            if dbg is not None and kt == 0:
                vd = sb.tile([128, 390], F32, name="vd", tag="vd")
                nc.vector.tensor_copy(vd[:], v_sb[0][:])
                nc.sync.dma_start(out=dbg["dbg_v0"][:], in_=vd[:])

        qt_tiles = {}
        kt_tiles = {}

        def emit_qkv_group(p, which, qb):
            """One accumulation group: 512 columns of Q^T or K^T for pair p."""
            w_sb, store, nm = (
                (wq_sb, qt_tiles, "qt") if which == 0 else (wk_sb, kt_tiles, "kt")
            )
            if qb == 0:
                store[p] = sb.tile([128, N], dt, name=f"{nm}{p}", tag=nm, bufs=wbufs)
            acc = ps.tile([128, 512], F32, name="dr", tag="dr", bufs=2)
            for ch in range(CH):
                nc.tensor.matmul(
                    acc[:],
                    lhsT=w_sb[ch][:, p * 128:(p + 1) * 128],
                    rhs=xt_sb[ch][:, qb * 512:(qb + 1) * 512],
                    start=(ch == 0), stop=(ch == CH - 1),
                )
            nc.vector.tensor_copy(store[p][:, qb * 512:(qb + 1) * 512], acc[:])

        def emit_proj_group(tt):
            tsl = slice(tt * 128, (tt + 1) * 128)
            pp = ps.tile([128, 1024], F32, name="s", tag="s", bufs=2)
            for ch in range(PAIRS):
                nc.tensor.matmul(
                    pp[:, 0:512], lhsT=ot_sb[ch][:, tsl], rhs=wp_sb[ch][:, 0:512],
                    start=(ch == 0), stop=(ch == PAIRS - 1),
                )
                nc.tensor.matmul(
                    pp[:, 512:768], lhsT=ot_sb[ch][:, tsl], rhs=wp_sb[ch][:, 512:768],
                    start=(ch == 0), stop=(ch == PAIRS - 1),
                )
            st = sb.tile([128, 768], F32, name="st", tag="st", bufs=2)
            nc.vector.tensor_copy(st[:], pp[:, 0:768])
            nc.sync.dma_start(out=out[tsl, :], in_=st[:])

        def emit_attn_qb(p, qb):
            qt_t = qt_tiles[p]
            kt_t = kt_tiles[p]
            qsl = slice(qb * 512, (qb + 1) * 512)
            u_a = ps.tile([128, 512], F32, name="ua", tag="u", bufs=2)
            u_b = ps.tile([128, 512], F32, name="ub", tag="u", bufs=2)
            for kt in range(KT):
                ksl = slice(kt * 128, (kt + 1) * 128)
                s_ps = ps.tile([128, 1024], F32, name="s", tag="s", bufs=2)
                # scores S^T for both heads, row-tiled (contract=64 each)
                nc.tensor.matmul(
                    s_ps[:, 0:512],
                    lhsT=kt_t[0:64, ksl], rhs=qt_t[0:64, qsl],
                    start=True, stop=True,
                )
                nc.tensor.matmul(
                    s_ps[:, 512:1024],
                    lhsT=kt_t[64:128, ksl], rhs=qt_t[64:128, qsl],
                    start=True, stop=True,
                )
                e_sb = sb.tile([128, 1024], dt, name="e", tag="e", bufs=3)
                nc.scalar.activation(e_sb[:], s_ps[:], EXP, scale=SCALE)
                if dbg is not None and p == 0 and qb == 0 and kt == 0:
                    ed = sb.tile([128, 1024], F32, name="ed", tag="ed")
                    nc.vector.tensor_copy(ed[:], e_sb[:])
                    nc.sync.dma_start(out=dbg["dbg_e0"][:], in_=ed[:])
                first = kt == 0
                last = kt == KT - 1
                # PV with the ones column: U[0:64] = P@V, U[64] = denominator
                nc.tensor.matmul(
                    u_a[0:65, :],
                    lhsT=v_sb[kt][:, (2 * p) * 65:(2 * p) * 65 + 65],
                    rhs=e_sb[:, 0:512],
                    start=first, stop=last,
                )
                nc.tensor.matmul(
                    u_b[0:65, :],
                    lhsT=v_sb[kt][:, (2 * p + 1) * 65:(2 * p + 1) * 65 + 65],
                    rhs=e_sb[:, 512:1024],
                    start=first, stop=last,
                )
            # normalization tail. Cheap copies first so the U PSUM banks free
            # up quickly; denominators are replicated across partitions via PE
            # FIRST (raw), then one fast reciprocal on the [128,512] tile runs
            # off the PE critical path.
            dsb = sb.tile([65, 1024], dt, name="dsb", tag="dsb", bufs=2)
            nc.vector.tensor_copy(dsb[64:65, 0:512], u_a[64:65, :])
            nc.vector.tensor_copy(dsb[64:65, 512:1024], u_b[64:65, :])
            ua_sb = sb.tile([64, 512], dt, name="uasb", tag="uasb", bufs=2)
            nc.vector.tensor_copy(ua_sb[:], u_a[0:64, :])
            tmp = sb.tile([64, 512], dt, name="tmp", tag="tmp", bufs=2)
            nc.vector.tensor_copy(tmp[:], u_b[0:64, :])
            if dbg is not None and p == 0 and qb == 0:
                nc.sync.dma_start(out=dbg["dbg_den"][:], in_=dsb[64:65, :])
            # replicate raw denominator rows across partitions via PE
            r_ps = ps.tile([128, 512], F32, name="dr", tag="dr", bufs=2)
            nc.tensor.matmul(
                r_ps[0:64, :], lhsT=ones_sb[64:65, 0:64], rhs=dsb[64:65, 0:512],
                start=True, stop=True,
            )
            nc.tensor.matmul(
                r_ps[64:128, :], lhsT=ones_sb[64:65, 0:64], rhs=dsb[64:65, 512:1024],
                start=True, stop=True,
            )
            rsb = sb.tile([128, 512], F32, name="rsb", tag="rsb", bufs=2)
            nc.vector.reciprocal_approx_fast(out=rsb[:], in_=r_ps[:])
            # head 2's U must move to partitions 64-127: PE shift via identity
            o2 = ps.tile([128, 512], F32, name="dr", tag="dr", bufs=2)
            nc.tensor.matmul(
                o2[64:128, :], lhsT=ident[0:64, 0:64], rhs=tmp[:],
                start=True, stop=True,
            )
            nc.vector.tensor_mul(ot_sb[p][0:64, qsl], ua_sb[:], rsb[0:64, :])
            nc.vector.tensor_mul(ot_sb[p][64:128, qsl], o2[64:128, :], rsb[64:128, :])

        # ---- schedule ----------------------------------------------------
        if overlap:
            # qkv(0) upfront; qkv(p+1) / proj interleaved at qb boundaries so
            # the in-order PE stream has DMA/ACT-independent work to chew on.
            for which in (0, 1):
                for qb in range(QB):
                    emit_qkv_group(0, which, qb)
            for p in range(PAIRS):
                for qb in range(QB):
                    emit_attn_qb(p, qb)
                    if p < PAIRS - 1:
                        emit_qkv_group(p + 1, qb // 2, qb % 2 * 2)
                        emit_qkv_group(p + 1, qb // 2, qb % 2 * 2 + 1)
                    else:
                        for tt in range(4 * qb, 4 * qb + 4):
                            emit_proj_group(tt)
        else:
            for p in range(PAIRS):
                for which in (0, 1):
                    for qb in range(QB):
                        emit_qkv_group(p, which, qb)
                for qb in range(QB):
                    emit_attn_qb(p, qb)
            for tt in range(KT):
                emit_proj_group(tt)


_NC = {}


def _get_nc(dtype, overlap=None):
    key = (dtype, overlap)
    if key not in _NC:
        _NC[key] = build_program(dtype, overlap=overlap)
    return _NC[key]


def make_in_maps(x, w_qkv, w_proj, dtype):
    np_dt = np.float32 if dtype == "f32" else ml_dtypes.bfloat16
    in_maps = []
    for c in range(NCORES):
        b = c // 2
        h0 = (c % 2) * 6 * HD
        in_maps.append({
            "xt": np.ascontiguousarray(x[b].T).astype(np_dt),
            "wq": np.ascontiguousarray(w_qkv[:, h0:h0 + 384]).astype(np_dt),
            "wk": np.ascontiguousarray(w_qkv[:, DIM + h0:DIM + h0 + 384]).astype(np_dt),
            "wv": np.ascontiguousarray(w_qkv[:, 2 * DIM + h0:2 * DIM + h0 + 384]).astype(np_dt),
            "wp": np.ascontiguousarray(w_proj[h0:h0 + 384, :]).astype(np_dt),
        })
    return in_maps


def run(x, w_qkv, w_proj, b_proj, trace=False, dtype=None, overlap=None):
    dtype = dtype or DEFAULT_DTYPE
    x = np.asarray(x, dtype=np.float32)
    w_qkv = np.asarray(w_qkv, dtype=np.float32)
    w_proj = np.asarray(w_proj, dtype=np.float32)
    b_proj = np.asarray(b_proj, dtype=np.float32)

    in_maps = make_in_maps(x, w_qkv, w_proj, dtype)
    res = run_bass_kernel_spmd(_get_nc(dtype, overlap), in_maps, list(range(NCORES)),
                               trace=trace)
    full = np.empty((B, N, DIM), dtype=np.float32)
    for b in range(B):
        full[b] = res.results[2 * b]["out"] + res.results[2 * b + 1]["out"] + b_proj
    return full, res


def kernel(x, w_qkv, w_proj, b_proj):
    full, _ = run(x, w_qkv, w_proj, b_proj, trace=False)
    return full



# revision 10
# speedup vs baseline: 2.6209x; 2.6209x over previous
"""Multi-head attention (B=4, N=2048, DIM=768, H=12) on 8 TRN2 cores.

Sharding: core c -> batch c//2, heads (c%2)*6 .. +6  (6 heads = 3 pairs).
Each core computes its heads' attention and a partial output projection
(row-sharded w_proj); host sums the two partials per batch and adds bias.

Per-core dataflow:
  inputs : xt [768,2048] (= x[b].T), wq/wk/wv [768,384], wp [384,768]
  qkv    : Q^T,K^T per head-pair [128,2048] (d-major), V token-major with a
           ones column appended per head ([64 V | 1] x 6 -> [128, 390])
  scores : S^T[keys, q], 2 heads row-tiled per key-tile matmul
  softmax: exp on ACT (scale folded in; max-subtraction skipped - scores O(5));
           PV with M=65 makes row 64 of U the denominator for free;
           normalization via fast reciprocal + PE-replicated recip rows
  PV     : U^T[d+1, q] accumulated over key chunks (V' chunks stationary)
  proj   : partial = OT.T-slices @ wp  (OT is d-major already - no transposes)

Matmul dtype is configurable: fp32 is exact but runs LOW_HIGH double-pass at
2 cyc/col per pass on the PE; bf16 is 4x faster per matmul.
"""

import sys

for _p in ("/opt/trn_rl_repo",):
    if _p not in sys.path:
        sys.path.insert(0, _p)

import numpy as np
import ml_dtypes

import concourse.bass as bass
import concourse.bacc as bacc
import concourse.mybir as mybir
import concourse.tile as tile
from concourse.bass_utils import run_bass_kernel_spmd
from concourse.masks import make_identity

DIM = 768
HEADS = 12
HD = 64
B = 4
N = 2048
NCORES = 8
PAIRS = 3          # head-pairs per core (6 heads)
CH = DIM // 128    # 6 contraction chunks of 128
KT = N // 128      # 16 key tiles
QB = N // 512      # 4 query blocks of 512
F32 = mybir.dt.float32
EXP = mybir.ActivationFunctionType.Exp
SCALE = HD ** -0.5

DEFAULT_DTYPE = "bf16"


def build_program(dtype="f32", debug=False, overlap=None):
    if overlap is None:
        overlap = dtype != "f32"
    dt = F32 if dtype == "f32" else mybir.dt.bfloat16
    nc = bacc.Bacc()
    xt = nc.declare_dram_parameter("xt", [DIM, N], dt, isOutput=False)
    wq = nc.declare_dram_parameter("wq", [DIM, PAIRS * 128], dt, isOutput=False)
    wk = nc.declare_dram_parameter("wk", [DIM, PAIRS * 128], dt, isOutput=False)
    wv = nc.declare_dram_parameter("wv", [DIM, PAIRS * 128], dt, isOutput=False)
    wp = nc.declare_dram_parameter("wp", [PAIRS * 128, DIM], dt, isOutput=False)
    out = nc.declare_dram_parameter("out", [N, DIM], F32, isOutput=True)
    dbg = None
    if debug:
        dbg = {
            "dbg_v0": nc.declare_dram_parameter("dbg_v0", [128, 390], F32, isOutput=True),
            "dbg_den": nc.declare_dram_parameter("dbg_den", [1, 1024], F32, isOutput=True),
            "dbg_dsb": nc.declare_dram_parameter("dbg_dsb", [1, 1024], F32, isOutput=True),
            "dbg_e0": nc.declare_dram_parameter("dbg_e0", [128, 1024], F32, isOutput=True),
        }

    with tile.TileContext(nc) as tc:
        emit(tc, nc, xt, wq, wk, wv, wp, out, dt, overlap=overlap, dbg=dbg)
    nc.compile()
    return nc


def emit(tc, nc, xt, wq, wk, wv, wp, out, dt, overlap, dbg=None):
    import contextlib

    ctx = contextlib.ExitStack()
    wbufs = 2 if overlap else 1
    with ctx:
        sb = ctx.enter_context(tc.tile_pool(name="sb", bufs=1))
        ps = ctx.enter_context(tc.tile_pool(name="ps", bufs=1, space="PSUM"))

        # ---- load inputs -------------------------------------------------
        # DMA order matches first-consumer order: wq/wk (small) first, then
        # xt split into qb-column blocks so qkv groups unblock progressively,
        # then wv/wp. Tile tracks deps at slice granularity.
        def load_w(dram, nm):
            tiles = []
            for ch in range(CH):
                t = sb.tile([128, PAIRS * 128], dt, name=f"{nm}{ch}", tag=f"{nm}{ch}")
                nc.sync.dma_start(out=t[:], in_=dram[ch * 128:(ch + 1) * 128, :])
                tiles.append(t)
            return tiles

        wq_sb = load_w(wq, "wq")
        wk_sb = load_w(wk, "wk")
        wv_sb = load_w(wv, "wv")

        xt_sb = [sb.tile([128, N], dt, name=f"xt{ch}", tag=f"xt{ch}")
                 for ch in range(CH)]
        for qb in range(QB):
            csl = slice(qb * 512, (qb + 1) * 512)
            for ch in range(CH):
                nc.sync.dma_start(out=xt_sb[ch][:, csl],
                                  in_=xt[ch * 128:(ch + 1) * 128, csl])

        wp_sb = []
        for ch in range(PAIRS):
            t = sb.tile([128, DIM], dt, name=f"wp{ch}", tag=f"wp{ch}")
            nc.sync.dma_start(out=t[:], in_=wp[ch * 128:(ch + 1) * 128, :])
            wp_sb.append(t)

        ones_sb = sb.tile([128, 64], dt, name="ones", tag="ones")
        nc.vector.memset(ones_sb[:], 1.0)
        ident = sb.tile([128, 128], dt, name="ident", tag="ident")
        make_identity(nc, ident)

        # e/v (softmax weights and values) use fp16: exp output is in
        # [0, ~e^5] where fp16 beats bf16 precision, and it matmuls at the
        # same 1 cyc/col.
        edt = mybir.dt.float16 if dt != F32 else F32

        # persistent SBUF tensors
        # v' layout per head g: cols [g*65 .. g*65+63] = V, col g*65+64 = 1.0
        v_sb = [sb.tile([128, 6 * 65], edt, name=f"v{k}", tag=f"v{k}")
                for k in range(KT)]
        ot_sb = [sb.tile([128, N], dt, name=f"ot{p}", tag=f"ot{p}")
                 for p in range(PAIRS)]

        # ---- V' = [x @ wv | 1] (token-major) -----------------------------
        def emit_v(kt):
            pv = ps.tile([128, 512], F32, name="dr", tag="dr", bufs=2)
            for ch in range(CH):
                nc.tensor.matmul(
                    pv[:, :PAIRS * 128],
                    lhsT=xt_sb[ch][:, kt * 128:(kt + 1) * 128],
                    rhs=wv_sb[ch][:],
                    start=(ch == 0), stop=(ch == CH - 1),
                )
            v3 = v_sb[kt].rearrange("p (g c) -> p g c", c=65)
            p3 = pv[:, :PAIRS * 128].rearrange("p (g c) -> p g c", c=64)
            nc.vector.tensor_copy(v3[:, :, 0:64], p3[:])
            nc.vector.memset(v3[:, :, 64:65], 1.0)

        qt_tiles = {}
        kt_tiles = {}

        def emit_qkv_group(p, which, qb):
            """One accumulation group: 512 columns of Q^T or K^T for pair p."""
            w_sb, store, nm = (
                (wq_sb, qt_tiles, "qt") if which == 0 else (wk_sb, kt_tiles, "kt")
            )
            if qb == 0:
                store[p] = sb.tile([128, N], dt, name=f"{nm}{p}", tag=nm, bufs=wbufs)
            acc = ps.tile([128, 512], F32, name="dr", tag="dr", bufs=2)
            for ch in range(CH):
                nc.tensor.matmul(
                    acc[:],
                    lhsT=w_sb[ch][:, p * 128:(p + 1) * 128],
                    rhs=xt_sb[ch][:, qb * 512:(qb + 1) * 512],
                    start=(ch == 0), stop=(ch == CH - 1),
                )
            nc.vector.tensor_copy(store[p][:, qb * 512:(qb + 1) * 512], acc[:])

        def emit_proj_group(tt):
            tsl = slice(tt * 128, (tt + 1) * 128)
            pp = ps.tile([128, 1024], F32, name="s", tag="s", bufs=2)
            for ch in range(PAIRS):
                nc.tensor.matmul(
                    pp[:, 0:512], lhsT=ot_sb[ch][:, tsl], rhs=wp_sb[ch][:, 0:512],
                    start=(ch == 0), stop=(ch == PAIRS - 1),
                )
                nc.tensor.matmul(
                    pp[:, 512:768], lhsT=ot_sb[ch][:, tsl], rhs=wp_sb[ch][:, 512:768],
                    start=(ch == 0), stop=(ch == PAIRS - 1),
                )
            st = sb.tile([128, 768], F32, name="st", tag="st", bufs=2)
            nc.vector.tensor_copy(st[:], pp[:, 0:768])
            nc.sync.dma_start(out=out[tsl, :], in_=st[:])

        def emit_attn_qb(p, qb):
            qt_t = qt_tiles[p]
            kt_t = kt_tiles[p]
            qsl = slice(qb * 512, (qb + 1) * 512)
            u_a = ps.tile([128, 512], F32, name="ua", tag="u", bufs=2)
            u_b = ps.tile([128, 512], F32, name="ub", tag="u", bufs=2)
            for kt in range(KT):
                ksl = slice(kt * 128, (kt + 1) * 128)
                s_ps = ps.tile([128, 1024], F32, name="s", tag="s", bufs=2)
                # scores S^T for both heads, row-tiled (contract=64 each)
                nc.tensor.matmul(
                    s_ps[:, 0:512],
                    lhsT=kt_t[0:64, ksl], rhs=qt_t[0:64, qsl],
                    start=True, stop=True,
                )
                nc.tensor.matmul(
                    s_ps[:, 512:1024],
                    lhsT=kt_t[64:128, ksl], rhs=qt_t[64:128, qsl],
                    start=True, stop=True,
                )
                e_sb = sb.tile([128, 1024], edt, name="e", tag="e", bufs=3)
                nc.scalar.activation(e_sb[:], s_ps[:], EXP, scale=SCALE)
                if dbg is not None and p == 0 and qb == 0 and kt == 0:
                    ed = sb.tile([128, 1024], F32, name="ed", tag="ed")
                    nc.vector.tensor_copy(ed[:], e_sb[:])
                    nc.sync.dma_start(out=dbg["dbg_e0"][:], in_=ed[:])
                first = kt == 0
                last = kt == KT - 1
                # PV with the ones column: U[0:64] = P@V, U[64] = denominator
                nc.tensor.matmul(
                    u_a[0:65, :],
                    lhsT=v_sb[kt][:, (2 * p) * 65:(2 * p) * 65 + 65],
                    rhs=e_sb[:, 0:512],
                    start=first, stop=last,
                )
                nc.tensor.matmul(
                    u_b[0:65, :],
                    lhsT=v_sb[kt][:, (2 * p + 1) * 65:(2 * p + 1) * 65 + 65],
                    rhs=e_sb[:, 512:1024],
                    start=first, stop=last,
                )
            # normalization tail. Cheap copies first so the U PSUM banks free
            # up quickly; denominators are replicated across partitions via PE
            # FIRST (raw), then one fast reciprocal on the [128,512] tile runs
            # off the PE critical path.
            dsb = sb.tile([65, 1024], dt, name="dsb", tag="dsb", bufs=2)
            nc.vector.tensor_copy(dsb[64:65, 0:512], u_a[64:65, :])
            nc.vector.tensor_copy(dsb[64:65, 512:1024], u_b[64:65, :])
            ua_sb = sb.tile([64, 512], dt, name="uasb", tag="uasb", bufs=2)
            nc.vector.tensor_copy(ua_sb[:], u_a[0:64, :])
            tmp = sb.tile([64, 512], dt, name="tmp", tag="tmp", bufs=2)
            nc.vector.tensor_copy(tmp[:], u_b[0:64, :])
            if dbg is not None and p == 0 and qb == 0:
                nc.sync.dma_start(out=dbg["dbg_den"][:], in_=dsb[64:65, :])
            # replicate raw denominator rows across partitions via PE
            r_ps = ps.tile([128, 512], F32, name="dr", tag="dr", bufs=2)
            nc.tensor.matmul(
                r_ps[0:64, :], lhsT=ones_sb[64:65, 0:64], rhs=dsb[64:65, 0:512],
                start=True, stop=True,
            )
            nc.tensor.matmul(
                r_ps[64:128, :], lhsT=ones_sb[64:65, 0:64], rhs=dsb[64:65, 512:1024],
                start=True, stop=True,
            )
            rsb = sb.tile([128, 512], F32, name="rsb", tag="rsb", bufs=2)
            nc.vector.reciprocal_approx_fast(out=rsb[:], in_=r_ps[:])
            # head 2's U must move to partitions 64-127: PE shift via identity
            o2 = ps.tile([128, 512], F32, name="dr", tag="dr", bufs=2)
            nc.tensor.matmul(
                o2[64:128, :], lhsT=ident[0:64, 0:64], rhs=tmp[:],
                start=True, stop=True,
            )
            nc.vector.tensor_mul(ot_sb[p][0:64, qsl], ua_sb[:], rsb[0:64, :])
            nc.vector.tensor_mul(ot_sb[p][64:128, qsl], o2[64:128, :], rsb[64:128, :])

        # ---- schedule ----------------------------------------------------
        if overlap:
            # qkv(0) upfront (its inputs arrive first), then V', then
            # qkv(p+1) / proj interleaved at qb boundaries so the in-order
            # PE stream has DMA/ACT-independent work to chew on.
            for which in (0, 1):
                for qb in range(QB):
                    emit_qkv_group(0, which, qb)
            for kt in range(KT):
                emit_v(kt)
            for p in range(PAIRS):
                for qb in range(QB):
                    emit_attn_qb(p, qb)
                    if p < PAIRS - 1:
                        emit_qkv_group(p + 1, qb // 2, qb % 2 * 2)
                        emit_qkv_group(p + 1, qb // 2, qb % 2 * 2 + 1)
                    else:
                        for tt in range(4 * qb, 4 * qb + 4):
                            emit_proj_group(tt)
        else:
            for kt in range(KT):
                emit_v(kt)
            for p in range(PAIRS):
                for which in (0, 1):
                    for qb in range(QB):
                        emit_qkv_group(p, which, qb)
                for qb in range(QB):
                    emit_attn_qb(p, qb)
            for tt in range(KT):
                emit_proj_group(tt)


_NC = {}


def _get_nc(dtype, overlap=None):
    key = (dtype, overlap)
    if key not in _NC:
        _NC[key] = build_program(dtype, overlap=overlap)
    return _NC[key]


def make_in_maps(x, w_qkv, w_proj, dtype):
    np_dt = np.float32 if dtype == "f32" else ml_dtypes.bfloat16
    in_maps = []
    for c in range(NCORES):
        b = c // 2
        h0 = (c % 2) * 6 * HD
        in_maps.append({
            "xt": np.ascontiguousarray(x[b].T).astype(np_dt),
            "wq": np.ascontiguousarray(w_qkv[:, h0:h0 + 384]).astype(np_dt),
            "wk": np.ascontiguousarray(w_qkv[:, DIM + h0:DIM + h0 + 384]).astype(np_dt),
            "wv": np.ascontiguousarray(w_qkv[:, 2 * DIM + h0:2 * DIM + h0 + 384]).astype(np_dt),
            "wp": np.ascontiguousarray(w_proj[h0:h0 + 384, :]).astype(np_dt),
        })
    return in_maps


def run(x, w_qkv, w_proj, b_proj, trace=False, dtype=None, overlap=None):
    dtype = dtype or DEFAULT_DTYPE
    x = np.asarray(x, dtype=np.float32)
    w_qkv = np.asarray(w_qkv, dtype=np.float32)
    w_proj = np.asarray(w_proj, dtype=np.float32)
    b_proj = np.asarray(b_proj, dtype=np.float32)

    in_maps = make_in_maps(x, w_qkv, w_proj, dtype)
    res = run_bass_kernel_spmd(_get_nc(dtype, overlap), in_maps, list(range(NCORES)),
                               trace=trace)
    full = np.empty((B, N, DIM), dtype=np.float32)
    for b in range(B):
        full[b] = res.results[2 * b]["out"] + res.results[2 * b + 1]["out"] + b_proj
    return full, res


def kernel(x, w_qkv, w_proj, b_proj):
    full, _ = run(x, w_qkv, w_proj, b_proj, trace=False)
    return full



# revision 16
# speedup vs baseline: 2.6882x; 1.0257x over previous
"""Multi-head attention (B=4, N=2048, DIM=768, H=12) on 8 TRN2 cores.

Sharding: core c -> batch c//2, heads (c%2)*6 .. +6  (6 heads = 3 pairs).
Each core computes its heads' attention and a partial output projection
(row-sharded w_proj); host sums the two partials per batch and adds bias.

Per-core dataflow:
  inputs : xt [768,2048] (= x[b].T), wq/wk/wv [768,384], wp [384,768]
  qkv    : Q^T,K^T per head-pair [128,2048] (d-major), V token-major with a
           ones column appended per head ([64 V | 1] x 6 -> [128, 390])
  scores : S^T[keys, q], 2 heads row-tiled per key-tile matmul
  softmax: exp on ACT (scale folded in; max-subtraction skipped - scores O(5));
           PV with M=65 makes row 64 of U the denominator for free;
           normalization via fast reciprocal + PE-replicated recip rows
  PV     : U^T[d+1, q] accumulated over key chunks (V' chunks stationary)
  proj   : partial = OT.T-slices @ wp  (OT is d-major already - no transposes)

Matmul dtype is configurable: fp32 is exact but runs LOW_HIGH double-pass at
2 cyc/col per pass on the PE; bf16 is 4x faster per matmul.
"""

import sys

for _p in ("/opt/trn_rl_repo",):
    if _p not in sys.path:
        sys.path.insert(0, _p)

import numpy as np
import ml_dtypes

import concourse.bass as bass
import concourse.bacc as bacc
import concourse.mybir as mybir
import concourse.tile as tile
from concourse.bass_utils import run_bass_kernel_spmd
from concourse.masks import make_identity

DIM = 768
HEADS = 12
HD = 64
B = 4
N = 2048
NCORES = 8
PAIRS = 3          # head-pairs per core (6 heads)
CH = DIM // 128    # 6 contraction chunks of 128
KT = N // 128      # 16 key tiles
QB = N // 512      # 4 query blocks of 512
F32 = mybir.dt.float32
EXP = mybir.ActivationFunctionType.Exp
SCALE = HD ** -0.5

DEFAULT_DTYPE = "bf16"


def build_program(dtype="f32", debug=False, overlap=None):
    if overlap is None:
        overlap = dtype != "f32"
    dt = F32 if dtype == "f32" else mybir.dt.bfloat16
    nc = bacc.Bacc()
    xt = nc.declare_dram_parameter("xt", [DIM, N], dt, isOutput=False)
    wq = nc.declare_dram_parameter("wq", [DIM, PAIRS * 128], dt, isOutput=False)
    wk = nc.declare_dram_parameter("wk", [DIM, PAIRS * 128], dt, isOutput=False)
    wv = nc.declare_dram_parameter("wv", [DIM, PAIRS * 128], dt, isOutput=False)
    wp = nc.declare_dram_parameter("wp", [PAIRS * 128, DIM], dt, isOutput=False)
    out = nc.declare_dram_parameter("out", [N, DIM], dt, isOutput=True)
    dbg = None
    if debug:
        dbg = {
            "dbg_v0": nc.declare_dram_parameter("dbg_v0", [128, 390], F32, isOutput=True),
            "dbg_den": nc.declare_dram_parameter("dbg_den", [1, 1024], F32, isOutput=True),
            "dbg_dsb": nc.declare_dram_parameter("dbg_dsb", [1, 1024], F32, isOutput=True),
            "dbg_e0": nc.declare_dram_parameter("dbg_e0", [128, 1024], F32, isOutput=True),
        }

    with tile.TileContext(nc) as tc:
        emit(tc, nc, xt, wq, wk, wv, wp, out, dt, overlap=overlap, dbg=dbg)
    nc.compile()
    return nc


def emit(tc, nc, xt, wq, wk, wv, wp, out, dt, overlap, dbg=None):
    import contextlib

    ctx = contextlib.ExitStack()
    wbufs = 2 if overlap else 1
    with ctx:
        sb = ctx.enter_context(tc.tile_pool(name="sb", bufs=1))
        ps = ctx.enter_context(tc.tile_pool(name="ps", bufs=1, space="PSUM"))

        # ---- load inputs -------------------------------------------------
        # DMA order matches first-consumer order: wq/wk (small) first, then
        # xt split into qb-column blocks so qkv groups unblock progressively,
        # then wv/wp. Tile tracks deps at slice granularity.
        def load_w(dram, nm):
            tiles = []
            for ch in range(CH):
                t = sb.tile([128, PAIRS * 128], dt, name=f"{nm}{ch}", tag=f"{nm}{ch}")
                nc.sync.dma_start(out=t[:], in_=dram[ch * 128:(ch + 1) * 128, :])
                tiles.append(t)
            return tiles

        wq_sb = load_w(wq, "wq")
        wk_sb = load_w(wk, "wk")

        xt_sb = [sb.tile([128, N], dt, name=f"xt{ch}", tag=f"xt{ch}")
                 for ch in range(CH)]

        def load_xt(qb):
            csl = slice(qb * 512, (qb + 1) * 512)
            for ch in range(CH):
                nc.sync.dma_start(out=xt_sb[ch][:, csl],
                                  in_=xt[ch * 128:(ch + 1) * 128, csl])

        load_xt(0)
        wv_sb = load_w(wv, "wv")
        for qb in range(1, QB):
            load_xt(qb)

        wp_sb = []
        for ch in range(PAIRS):
            t = sb.tile([128, DIM], dt, name=f"wp{ch}", tag=f"wp{ch}")
            nc.sync.dma_start(out=t[:], in_=wp[ch * 128:(ch + 1) * 128, :])
            wp_sb.append(t)

        ones_sb = sb.tile([128, 64], dt, name="ones", tag="ones")
        nc.vector.memset(ones_sb[:], 1.0)
        ident = sb.tile([128, 128], dt, name="ident", tag="ident")
        make_identity(nc, ident)

        # e/v (softmax weights and values) use fp16: exp output is in
        # [0, ~e^5] where fp16 beats bf16 precision, and it matmuls at the
        # same 1 cyc/col.
        edt = mybir.dt.float16 if dt != F32 else F32

        # persistent SBUF tensors
        # v' layout per head g: cols [g*65 .. g*65+63] = V, col g*65+64 = 1.0
        v_sb = [sb.tile([128, 6 * 65], edt, name=f"v{k}", tag=f"v{k}")
                for k in range(KT)]
        ot_sb = [sb.tile([128, N], dt, name=f"ot{p}", tag=f"ot{p}")
                 for p in range(PAIRS)]

        # ---- V' = [x @ wv | 1] (token-major) -----------------------------
        def emit_v(kt):
            pv = ps.tile([128, 512], F32, name="dr", tag="dr", bufs=2)
            for ch in range(CH):
                nc.tensor.matmul(
                    pv[:, :PAIRS * 128],
                    lhsT=xt_sb[ch][:, kt * 128:(kt + 1) * 128],
                    rhs=wv_sb[ch][:],
                    start=(ch == 0), stop=(ch == CH - 1),
                )
            v3 = v_sb[kt].rearrange("p (g c) -> p g c", c=65)
            p3 = pv[:, :PAIRS * 128].rearrange("p (g c) -> p g c", c=64)
            nc.vector.tensor_copy(v3[:, :, 0:64], p3[:])
            nc.vector.memset(v3[:, :, 64:65], 1.0)

        qt_tiles = {}
        kt_tiles = {}

        def emit_qkv_group(p, which, qb):
            """One accumulation group: 512 columns of Q^T or K^T for pair p."""
            w_sb, store, nm = (
                (wq_sb, qt_tiles, "qt") if which == 0 else (wk_sb, kt_tiles, "kt")
            )
            if qb == 0:
                store[p] = sb.tile([128, N], dt, name=f"{nm}{p}", tag=nm, bufs=wbufs)
            acc = ps.tile([128, 512], F32, name="dr", tag="dr", bufs=2)
            for ch in range(CH):
                nc.tensor.matmul(
                    acc[:],
                    lhsT=w_sb[ch][:, p * 128:(p + 1) * 128],
                    rhs=xt_sb[ch][:, qb * 512:(qb + 1) * 512],
                    start=(ch == 0), stop=(ch == CH - 1),
                )
            nc.vector.tensor_copy(store[p][:, qb * 512:(qb + 1) * 512], acc[:])

        def emit_proj_group(tt):
            tsl = slice(tt * 128, (tt + 1) * 128)
            pp = ps.tile([128, 1024], F32, name="s", tag="s", bufs=2)
            for ch in range(PAIRS):
                nc.tensor.matmul(
                    pp[:, 0:512], lhsT=ot_sb[ch][:, tsl], rhs=wp_sb[ch][:, 0:512],
                    start=(ch == 0), stop=(ch == PAIRS - 1),
                )
                nc.tensor.matmul(
                    pp[:, 512:768], lhsT=ot_sb[ch][:, tsl], rhs=wp_sb[ch][:, 512:768],
                    start=(ch == 0), stop=(ch == PAIRS - 1),
                )
            st = sb.tile([128, 768], dt, name="st", tag="st", bufs=2)
            nc.vector.tensor_copy(st[:], pp[:, 0:768])
            nc.sync.dma_start(out=out[tsl, :], in_=st[:])

        def attn_begin(p, qb):
            return {
                "p": p, "qb": qb,
                "qsl": slice(qb * 512, (qb + 1) * 512),
                "u_a": ps.tile([128, 512], F32, name="ua", tag="u", bufs=2),
                "u_b": ps.tile([128, 512], F32, name="ub", tag="u", bufs=2),
            }

        def attn_step(ast, kt):
            p, qsl = ast["p"], ast["qsl"]
            qt_t = qt_tiles[p]
            kt_t = kt_tiles[p]
            ksl = slice(kt * 128, (kt + 1) * 128)
            s_ps = ps.tile([128, 1024], F32, name="s", tag="s", bufs=2)
            # scores S^T for both heads, row-tiled (contract=64 each)
            nc.tensor.matmul(
                s_ps[:, 0:512],
                lhsT=kt_t[0:64, ksl], rhs=qt_t[0:64, qsl],
                start=True, stop=True,
            )
            nc.tensor.matmul(
                s_ps[:, 512:1024],
                lhsT=kt_t[64:128, ksl], rhs=qt_t[64:128, qsl],
                start=True, stop=True,
            )
            e_sb = sb.tile([128, 1024], edt, name="e", tag="e", bufs=3)
            nc.scalar.activation(e_sb[:], s_ps[:], EXP, scale=SCALE)
            first = kt == 0
            last = kt == KT - 1
            # PV with the ones column: U[0:64] = P@V, U[64] = denominator
            nc.tensor.matmul(
                ast["u_a"][0:65, :],
                lhsT=v_sb[kt][:, (2 * p) * 65:(2 * p) * 65 + 65],
                rhs=e_sb[:, 0:512],
                start=first, stop=last,
            )
            nc.tensor.matmul(
                ast["u_b"][0:65, :],
                lhsT=v_sb[kt][:, (2 * p + 1) * 65:(2 * p + 1) * 65 + 65],
                rhs=e_sb[:, 512:1024],
                start=first, stop=last,
            )

        def attn_end(ast):
            p, qb, qsl = ast["p"], ast["qb"], ast["qsl"]
            u_a, u_b = ast["u_a"], ast["u_b"]
            # normalization tail. Cheap copies first so the U PSUM banks free
            # up quickly; denominators are replicated across partitions via PE
            # FIRST (raw), then one fast reciprocal on the [128,512] tile runs
            # off the PE critical path.
            dsb = sb.tile([65, 1024], dt, name="dsb", tag="dsb", bufs=2)
            nc.vector.tensor_copy(dsb[64:65, 0:512], u_a[64:65, :])
            nc.vector.tensor_copy(dsb[64:65, 512:1024], u_b[64:65, :])
            ua_sb = sb.tile([64, 512], dt, name="uasb", tag="uasb", bufs=2)
            nc.vector.tensor_copy(ua_sb[:], u_a[0:64, :])
            tmp = sb.tile([64, 512], dt, name="tmp", tag="tmp", bufs=2)
            nc.vector.tensor_copy(tmp[:], u_b[0:64, :])
            if dbg is not None and p == 0 and qb == 0:
                nc.sync.dma_start(out=dbg["dbg_den"][:], in_=dsb[64:65, :])
            # replicate raw denominator rows across partitions via PE
            r_ps = ps.tile([128, 512], F32, name="dr", tag="dr", bufs=2)
            nc.tensor.matmul(
                r_ps[0:64, :], lhsT=ones_sb[64:65, 0:64], rhs=dsb[64:65, 0:512],
                start=True, stop=True,
            )
            nc.tensor.matmul(
                r_ps[64:128, :], lhsT=ones_sb[64:65, 0:64], rhs=dsb[64:65, 512:1024],
                start=True, stop=True,
            )
            rsb = sb.tile([128, 512], F32, name="rsb", tag="rsb", bufs=2)
            nc.vector.reciprocal_approx_fast(out=rsb[:], in_=r_ps[:])
            # head 2's U must move to partitions 64-127: PE shift via identity
            o2 = ps.tile([128, 512], F32, name="dr", tag="dr", bufs=2)
            nc.tensor.matmul(
                o2[64:128, :], lhsT=ident[0:64, 0:64], rhs=tmp[:],
                start=True, stop=True,
            )
            nc.vector.tensor_mul(ot_sb[p][0:64, qsl], ua_sb[:], rsb[0:64, :])
            nc.vector.tensor_mul(ot_sb[p][64:128, qsl], o2[64:128, :], rsb[64:128, :])

        def emit_attn_qb(p, qb):
            ast = attn_begin(p, qb)
            for kt in range(KT):
                attn_step(ast, kt)
            attn_end(ast)

        # ---- schedule ----------------------------------------------------
        if overlap:
            # Pipelined head: only Q0(qb0)/K0/V'(0:4) before attention
            # starts; the remaining V' tiles and Q0 groups are emitted inside
            # the first attention block so the ACT exp stream starts ~6us in.
            # Later pairs' qkv and the proj groups are interleaved at qb
            # boundaries so the in-order PE stream has DMA/ACT-independent
            # work to chew on.
            emit_qkv_group(0, 0, 0)
            for qb in range(QB):
                emit_qkv_group(0, 1, qb)
            for kt in range(4):
                emit_v(kt)
            ast0 = attn_begin(0, 0)
            for kt in range(KT):
                if kt + 4 < KT:
                    emit_v(kt + 4)
                attn_step(ast0, kt)
                if kt in (5, 9, 13):
                    emit_qkv_group(0, 0, (kt - 1) // 4)
            attn_end(ast0)
            # remaining attention; pair p+1's 8 qkv groups spread over the
            # available attn blocks of pair p.
            pend = [(1, w, qb) for w in (0, 1) for qb in range(QB)]
            for qb in range(1, QB):
                emit_attn_qb(0, qb)
                take, pend = pend[:3], pend[3:]
                for g in take:
                    emit_qkv_group(*g)
            for qb in range(QB):
                emit_attn_qb(1, qb)
                emit_qkv_group(2, qb // 2, qb % 2 * 2)
                emit_qkv_group(2, qb // 2, qb % 2 * 2 + 1)
            for qb in range(QB):
                emit_attn_qb(2, qb)
                for tt in range(4 * qb, 4 * qb + 4):
                    emit_proj_group(tt)
        else:
            for kt in range(KT):
                emit_v(kt)
            for p in range(PAIRS):
                for which in (0, 1):
                    for qb in range(QB):
                        emit_qkv_group(p, which, qb)
                for qb in range(QB):
                    emit_attn_qb(p, qb)
            for tt in range(KT):
                emit_proj_group(tt)


_NC = {}


def _get_nc(dtype, overlap=None):
    key = (dtype, overlap)
    if key not in _NC:
        _NC[key] = build_program(dtype, overlap=overlap)
    return _NC[key]


def make_in_maps(x, w_qkv, w_proj, dtype):
    np_dt = np.float32 if dtype == "f32" else ml_dtypes.bfloat16
    in_maps = []
    for c in range(NCORES):
        b = c // 2
        h0 = (c % 2) * 6 * HD
        in_maps.append({
            "xt": np.ascontiguousarray(x[b].T).astype(np_dt),
            "wq": np.ascontiguousarray(w_qkv[:, h0:h0 + 384]).astype(np_dt),
            "wk": np.ascontiguousarray(w_qkv[:, DIM + h0:DIM + h0 + 384]).astype(np_dt),
            "wv": np.ascontiguousarray(w_qkv[:, 2 * DIM + h0:2 * DIM + h0 + 384]).astype(np_dt),
            "wp": np.ascontiguousarray(w_proj[h0:h0 + 384, :]).astype(np_dt),
        })
    return in_maps


def run(x, w_qkv, w_proj, b_proj, trace=False, dtype=None, overlap=None):
    dtype = dtype or DEFAULT_DTYPE
    x = np.asarray(x, dtype=np.float32)
    w_qkv = np.asarray(w_qkv, dtype=np.float32)
    w_proj = np.asarray(w_proj, dtype=np.float32)
    b_proj = np.asarray(b_proj, dtype=np.float32)

    in_maps = make_in_maps(x, w_qkv, w_proj, dtype)
    res = run_bass_kernel_spmd(_get_nc(dtype, overlap), in_maps, list(range(NCORES)),
                               trace=trace)
    full = np.empty((B, N, DIM), dtype=np.float32)
    for b in range(B):
        full[b] = (res.results[2 * b]["out"].astype(np.float32)
                   + res.results[2 * b + 1]["out"].astype(np.float32) + b_proj)
    return full, res


def kernel(x, w_qkv, w_proj, b_proj):
    full, _ = run(x, w_qkv, w_proj, b_proj, trace=False)
    return full



# revision 19
# speedup vs baseline: 2.9802x; 1.1086x over previous
"""Multi-head attention (B=4, N=2048, DIM=768, H=12) on 8 TRN2 cores.

Sharding: core c -> batch c//2, heads (c%2)*6 .. +6  (6 heads = 3 pairs).
Each core computes its heads' attention and a partial output projection
(row-sharded w_proj); host sums the two partials per batch and adds bias.

Per-core dataflow:
  inputs : xt [768,2048] (= x[b].T), wq/wk/wv [768,384], wp [384,768]
  qkv    : Q^T,K^T per head-pair [128,2048] (d-major), V token-major with a
           ones column appended per head ([64 V | 1] x 6 -> [128, 390])
  scores : S^T[keys, q], 2 heads row-tiled per key-tile matmul
  softmax: exp on ACT (scale folded in; max-subtraction skipped - scores O(5));
           PV with M=65 makes row 64 of U the denominator for free;
           normalization via fast reciprocal + PE-replicated recip rows
  PV     : U^T[d+1, q] accumulated over key chunks (V' chunks stationary)
  proj   : partial = OT.T-slices @ wp  (OT is d-major already - no transposes)

Matmul dtype is configurable: fp32 is exact but runs LOW_HIGH double-pass at
2 cyc/col per pass on the PE; bf16 is 4x faster per matmul.
"""

import sys

for _p in ("/opt/trn_rl_repo",):
    if _p not in sys.path:
        sys.path.insert(0, _p)

import numpy as np
import ml_dtypes

import concourse.bass as bass
import concourse.bacc as bacc
import concourse.mybir as mybir
import concourse.tile as tile
from concourse.bass_utils import run_bass_kernel_spmd
from concourse.masks import make_identity

DIM = 768
HEADS = 12
HD = 64
B = 4
N = 2048
NCORES = 8
PAIRS = 3          # head-pairs per core (6 heads)
CH = DIM // 128    # 6 contraction chunks of 128
KT = N // 128      # 16 key tiles
QB = N // 512      # 4 query blocks of 512
F32 = mybir.dt.float32
EXP = mybir.ActivationFunctionType.Exp
SCALE = HD ** -0.5

DEFAULT_DTYPE = "bf16"


def build_program(dtype="f32", debug=False, overlap=None):
    if overlap is None:
        overlap = dtype != "f32"
    dt = F32 if dtype == "f32" else mybir.dt.bfloat16
    nc = bacc.Bacc()
    xt = nc.declare_dram_parameter("xt", [DIM, N], dt, isOutput=False)
    wq = nc.declare_dram_parameter("wq", [DIM, PAIRS * 128], dt, isOutput=False)
    wk = nc.declare_dram_parameter("wk", [DIM, PAIRS * 128], dt, isOutput=False)
    wv = nc.declare_dram_parameter("wv", [DIM, PAIRS * 128], dt, isOutput=False)
    wp = nc.declare_dram_parameter("wp", [PAIRS * 128, DIM], dt, isOutput=False)
    out = nc.declare_dram_parameter("out", [N, DIM], dt, isOutput=True)
    dbg = None
    if debug:
        dbg = {
            "dbg_v0": nc.declare_dram_parameter("dbg_v0", [128, 390], F32, isOutput=True),
            "dbg_den": nc.declare_dram_parameter("dbg_den", [1, 1024], F32, isOutput=True),
            "dbg_dsb": nc.declare_dram_parameter("dbg_dsb", [1, 1024], F32, isOutput=True),
            "dbg_e0": nc.declare_dram_parameter("dbg_e0", [128, 1024], F32, isOutput=True),
        }

    with tile.TileContext(nc) as tc:
        emit(tc, nc, xt, wq, wk, wv, wp, out, dt, overlap=overlap, dbg=dbg)
    nc.compile()
    return nc


def emit(tc, nc, xt, wq, wk, wv, wp, out, dt, overlap, dbg=None):
    import contextlib

    ctx = contextlib.ExitStack()
    wbufs = 2 if overlap else 1
    with ctx:
        sb = ctx.enter_context(tc.tile_pool(name="sb", bufs=1))
        ps = ctx.enter_context(tc.tile_pool(name="ps", bufs=1, space="PSUM"))

        # ---- load inputs -------------------------------------------------
        # DMA order matches first-consumer order: wq/wk (small) first, then
        # xt split into qb-column blocks so qkv groups unblock progressively,
        # then wv/wp. Tile tracks deps at slice granularity.
        def load_w(dram, nm):
            tiles = []
            for ch in range(CH):
                t = sb.tile([128, PAIRS * 128], dt, name=f"{nm}{ch}", tag=f"{nm}{ch}")
                nc.sync.dma_start(out=t[:], in_=dram[ch * 128:(ch + 1) * 128, :])
                tiles.append(t)
            return tiles

        wv_sb = load_w(wv, "wv")
        wq_sb = load_w(wq, "wq")
        wk_sb = load_w(wk, "wk")

        xt_sb = [sb.tile([128, N], dt, name=f"xt{ch}", tag=f"xt{ch}")
                 for ch in range(CH)]
        for qb in range(QB):
            csl = slice(qb * 512, (qb + 1) * 512)
            for ch in range(CH):
                nc.sync.dma_start(out=xt_sb[ch][:, csl],
                                  in_=xt[ch * 128:(ch + 1) * 128, csl])

        wp_sb = []
        for ch in range(PAIRS):
            t = sb.tile([128, DIM], dt, name=f"wp{ch}", tag=f"wp{ch}")
            nc.sync.dma_start(out=t[:], in_=wp[ch * 128:(ch + 1) * 128, :])
            wp_sb.append(t)

        ones_sb = sb.tile([128, 64], dt, name="ones", tag="ones")
        nc.vector.memset(ones_sb[:], 1.0)
        ident = sb.tile([128, 128], dt, name="ident", tag="ident")
        make_identity(nc, ident)

        # e/v (softmax weights and values) use fp16: exp output is in
        # [0, ~e^5] where fp16 beats bf16 precision, and it matmuls at the
        # same 1 cyc/col.
        edt = mybir.dt.float16 if dt != F32 else F32

        # persistent SBUF tensors
        # v' layout per head g: cols [g*65 .. g*65+63] = V, col g*65+64 = 1.0
        v_sb = [sb.tile([128, 6 * 65], edt, name=f"v{k}", tag=f"v{k}")
                for k in range(KT)]
        ot_sb = [sb.tile([128, N], dt, name=f"ot{p}", tag=f"ot{p}")
                 for p in range(PAIRS)]

        # ---- V' = [x @ wv | 1] (token-major) -----------------------------
        def emit_v(kt):
            pv = ps.tile([128, 512], F32, name="dr", tag="dr", bufs=2)
            for ch in range(CH):
                nc.tensor.matmul(
                    pv[:, :PAIRS * 128],
                    lhsT=xt_sb[ch][:, kt * 128:(kt + 1) * 128],
                    rhs=wv_sb[ch][:],
                    start=(ch == 0), stop=(ch == CH - 1),
                )
            v3 = v_sb[kt].rearrange("p (g c) -> p g c", c=65)
            p3 = pv[:, :PAIRS * 128].rearrange("p (g c) -> p g c", c=64)
            nc.vector.tensor_copy(v3[:, :, 0:64], p3[:])
            nc.vector.memset(v3[:, :, 64:65], 1.0)

        qt_tiles = {}
        kt_tiles = {}

        def emit_qkv_group(p, which, qb):
            """One accumulation group: 512 columns of Q^T or K^T for pair p."""
            w_sb, store, nm = (
                (wq_sb, qt_tiles, "qt") if which == 0 else (wk_sb, kt_tiles, "kt")
            )
            if qb == 0:
                store[p] = sb.tile([128, N], dt, name=f"{nm}{p}", tag=nm, bufs=wbufs)
            acc = ps.tile([128, 512], F32, name="dr", tag="dr", bufs=2)
            for ch in range(CH):
                nc.tensor.matmul(
                    acc[:],
                    lhsT=w_sb[ch][:, p * 128:(p + 1) * 128],
                    rhs=xt_sb[ch][:, qb * 512:(qb + 1) * 512],
                    start=(ch == 0), stop=(ch == CH - 1),
                )
            nc.vector.tensor_copy(store[p][:, qb * 512:(qb + 1) * 512], acc[:])

        def emit_proj_group(tt):
            # two dr-sized PSUM halves so proj can interleave inside attn
            # blocks without competing for the "s" slots the exp stream needs
            tsl = slice(tt * 128, (tt + 1) * 128)
            pa = ps.tile([128, 512], F32, name="dr", tag="dr", bufs=2)
            pb = ps.tile([128, 256], F32, name="dr2", tag="dr", bufs=2)
            for ch in range(PAIRS):
                nc.tensor.matmul(
                    pa[:], lhsT=ot_sb[ch][:, tsl], rhs=wp_sb[ch][:, 0:512],
                    start=(ch == 0), stop=(ch == PAIRS - 1),
                )
                nc.tensor.matmul(
                    pb[:], lhsT=ot_sb[ch][:, tsl], rhs=wp_sb[ch][:, 512:768],
                    start=(ch == 0), stop=(ch == PAIRS - 1),
                )
            st = sb.tile([128, 768], dt, name="st", tag="st", bufs=2)
            nc.vector.tensor_copy(st[:, 0:512], pa[:])
            nc.vector.tensor_copy(st[:, 512:768], pb[:])
            nc.sync.dma_start(out=out[tsl, :], in_=st[:])

        def attn_begin(p, qb):
            return {
                "p": p, "qb": qb,
                "qsl": slice(qb * 512, (qb + 1) * 512),
                "u_a": ps.tile([128, 512], F32, name="ua", tag="u", bufs=2),
                "u_b": ps.tile([128, 512], F32, name="ub", tag="u", bufs=2),
            }

        def attn_step(ast, kt):
            p, qsl = ast["p"], ast["qsl"]
            qt_t = qt_tiles[p]
            kt_t = kt_tiles[p]
            ksl = slice(kt * 128, (kt + 1) * 128)
            s_ps = ps.tile([128, 1024], F32, name="s", tag="s", bufs=2)
            # scores S^T for both heads, row-tiled (contract=64 each)
            nc.tensor.matmul(
                s_ps[:, 0:512],
                lhsT=kt_t[0:64, ksl], rhs=qt_t[0:64, qsl],
                start=True, stop=True,
            )
            nc.tensor.matmul(
                s_ps[:, 512:1024],
                lhsT=kt_t[64:128, ksl], rhs=qt_t[64:128, qsl],
                start=True, stop=True,
            )
            e_sb = sb.tile([128, 1024], edt, name="e", tag="e", bufs=3)
            nc.scalar.activation(e_sb[:], s_ps[:], EXP, scale=SCALE)
            first = kt == 0
            last = kt == KT - 1
            # PV with the ones column: U[0:64] = P@V, U[64] = denominator
            nc.tensor.matmul(
                ast["u_a"][0:65, :],
                lhsT=v_sb[kt][:, (2 * p) * 65:(2 * p) * 65 + 65],
                rhs=e_sb[:, 0:512],
                start=first, stop=last,
            )
            nc.tensor.matmul(
                ast["u_b"][0:65, :],
                lhsT=v_sb[kt][:, (2 * p + 1) * 65:(2 * p + 1) * 65 + 65],
                rhs=e_sb[:, 512:1024],
                start=first, stop=last,
            )

        def attn_end(ast):
            p, qb, qsl = ast["p"], ast["qb"], ast["qsl"]
            u_a, u_b = ast["u_a"], ast["u_b"]
            # normalization tail. Cheap copies first so the U PSUM banks free
            # up quickly; denominators are replicated across partitions via PE
            # FIRST (raw), then one fast reciprocal on the [128,512] tile runs
            # off the PE critical path.
            dsb = sb.tile([65, 1024], dt, name="dsb", tag="dsb", bufs=2)
            nc.vector.tensor_copy(dsb[64:65, 0:512], u_a[64:65, :])
            nc.vector.tensor_copy(dsb[64:65, 512:1024], u_b[64:65, :])
            ua_sb = sb.tile([64, 512], dt, name="uasb", tag="uasb", bufs=2)
            nc.vector.tensor_copy(ua_sb[:], u_a[0:64, :])
            tmp = sb.tile([64, 512], dt, name="tmp", tag="tmp", bufs=2)
            nc.vector.tensor_copy(tmp[:], u_b[0:64, :])
            if dbg is not None and p == 0 and qb == 0:
                nc.sync.dma_start(out=dbg["dbg_den"][:], in_=dsb[64:65, :])
            # replicate raw denominator rows across partitions via PE
            r_ps = ps.tile([128, 512], F32, name="dr", tag="dr", bufs=2)
            nc.tensor.matmul(
                r_ps[0:64, :], lhsT=ones_sb[64:65, 0:64], rhs=dsb[64:65, 0:512],
                start=True, stop=True,
            )
            nc.tensor.matmul(
                r_ps[64:128, :], lhsT=ones_sb[64:65, 0:64], rhs=dsb[64:65, 512:1024],
                start=True, stop=True,
            )
            rsb = sb.tile([128, 512], F32, name="rsb", tag="rsb", bufs=2)
            nc.vector.reciprocal_approx_fast(out=rsb[:], in_=r_ps[:])
            # head 2's U must move to partitions 64-127: PE shift via identity
            o2 = ps.tile([128, 512], F32, name="dr", tag="dr", bufs=2)
            nc.tensor.matmul(
                o2[64:128, :], lhsT=ident[0:64, 0:64], rhs=tmp[:],
                start=True, stop=True,
            )
            nc.vector.tensor_mul(ot_sb[p][0:64, qsl], ua_sb[:], rsb[0:64, :])
            nc.vector.tensor_mul(ot_sb[p][64:128, qsl], o2[64:128, :], rsb[64:128, :])

        def attn_block(p, qb, fillers=()):
            """One attention block with PE filler work woven BETWEEN kt
            steps, so fillers land in the PE's exp-wait slack instead of
            running as a bulk slug that starves the ACT stream."""
            fillers = list(fillers)
            nf = len(fillers)
            pos = [((i + 1) * KT) // (nf + 1) for i in range(nf)]
            ast = attn_begin(p, qb)
            fi = 0
            for kt in range(KT):
                attn_step(ast, kt)
                while fi < nf and pos[fi] <= kt:
                    fillers[fi]()
                    fi += 1
            attn_end(ast)
            while fi < nf:
                fillers[fi]()
                fi += 1

        def emit_attn_qb(p, qb):
            attn_block(p, qb)

        # ---- schedule ----------------------------------------------------
        if overlap:
            # Pipelined head: only Q0(qb0)/K0/V'(0:4) before attention
            # starts; the remaining V' tiles and Q0 groups are emitted inside
            # the first attention block so the ACT exp stream starts early.
            # Later pairs' qkv groups and the proj groups are woven between
            # the kt steps of subsequent attention blocks.
            emit_qkv_group(0, 0, 0)
            for qb in range(QB):
                emit_qkv_group(0, 1, qb)
            for kt in range(4):
                emit_v(kt)
            ast0 = attn_begin(0, 0)
            for kt in range(KT):
                if kt + 4 < KT:
                    emit_v(kt + 4)
                attn_step(ast0, kt)
                if kt in (5, 9, 13):
                    emit_qkv_group(0, 0, (kt - 1) // 4)
            attn_end(ast0)
            # pair p+1's 8 qkv groups spread over pair p's attn blocks
            g1 = [(1, w, qb) for w in (0, 1) for qb in range(QB)]
            for i, qb in enumerate(range(1, QB)):
                gs = g1[3 * i:3 * i + 3]
                attn_block(0, qb, [lambda g=g: emit_qkv_group(*g) for g in gs])
            g2 = [(2, w, qb) for w in (0, 1) for qb in range(QB)]
            for qb in range(QB):
                gs = g2[2 * qb:2 * qb + 2]
                attn_block(1, qb, [lambda g=g: emit_qkv_group(*g) for g in gs])
            attn_block(2, 0)
            for qb in range(1, QB):
                attn_block(2, qb, [lambda t=tt: emit_proj_group(t)
                                   for tt in range(4 * (qb - 1), 4 * qb)])
            for tt in range(4 * (QB - 1), 4 * QB):
                emit_proj_group(tt)
        else:
            for kt in range(KT):
                emit_v(kt)
            for p in range(PAIRS):
                for which in (0, 1):
                    for qb in range(QB):
                        emit_qkv_group(p, which, qb)
                for qb in range(QB):
                    emit_attn_qb(p, qb)
            for tt in range(KT):
                emit_proj_group(tt)


_NC = {}


def _get_nc(dtype, overlap=None):
    key = (dtype, overlap)
    if key not in _NC:
        _NC[key] = build_program(dtype, overlap=overlap)
    return _NC[key]


def make_in_maps(x, w_qkv, w_proj, dtype):
    np_dt = np.float32 if dtype == "f32" else ml_dtypes.bfloat16
    in_maps = []
    for c in range(NCORES):
        b = c // 2
        h0 = (c % 2) * 6 * HD
        in_maps.append({
            "xt": np.ascontiguousarray(x[b].T).astype(np_dt),
            "wq": np.ascontiguousarray(w_qkv[:, h0:h0 + 384]).astype(np_dt),
            "wk": np.ascontiguousarray(w_qkv[:, DIM + h0:DIM + h0 + 384]).astype(np_dt),
            "wv": np.ascontiguousarray(w_qkv[:, 2 * DIM + h0:2 * DIM + h0 + 384]).astype(np_dt),
            "wp": np.ascontiguousarray(w_proj[h0:h0 + 384, :]).astype(np_dt),
        })
    return in_maps


def run(x, w_qkv, w_proj, b_proj, trace=False, dtype=None, overlap=None):
    dtype = dtype or DEFAULT_DTYPE
    x = np.asarray(x, dtype=np.float32)
    w_qkv = np.asarray(w_qkv, dtype=np.float32)
    w_proj = np.asarray(w_proj, dtype=np.float32)
    b_proj = np.asarray(b_proj, dtype=np.float32)

    in_maps = make_in_maps(x, w_qkv, w_proj, dtype)
    res = run_bass_kernel_spmd(_get_nc(dtype, overlap), in_maps, list(range(NCORES)),
                               trace=trace)
    full = np.empty((B, N, DIM), dtype=np.float32)
    for b in range(B):
        full[b] = (res.results[2 * b]["out"].astype(np.float32)
                   + res.results[2 * b + 1]["out"].astype(np.float32) + b_proj)
    return full, res


def kernel(x, w_qkv, w_proj, b_proj):
    full, _ = run(x, w_qkv, w_proj, b_proj, trace=False)
    return full

